# revision 18
# baseline (speedup 1.0000x reference)
"""Trainium2 Bass kernel for the DocRED-style segment_reduce model.

Sharding: 8 cores, data-parallel: core c -> (doc = c//2, pair-half = c%2).
Each core independently computes logits for its 256 pairs. No collectives.
All segment reductions / gathers are lowered to one-hot matmuls whose
one-hot matrices are built on the host from the integer inputs and passed
as per-core input tensors (the SPMD program itself is index-agnostic).
"""

import os

import numpy as np

import concourse.bacc as bacc
import concourse.bass as bass
import concourse.mybir as mybir
import concourse.tile as tile
from concourse.bass_utils import run_bass_kernel_spmd

B, M, H = 4, 128, 1024
NH, L = 16, 1024
E, R = 64, 512
EMB, BS, NCL = 768, 64, 97
K12 = EMB // BS  # 12 blocks
NCORES = 8
RPC = R // 2  # pairs per core

F32 = mybir.dt.float32
F32R = mybir.dt.float32r
BF16 = mybir.dt.bfloat16

# matmul/compute dtype mode: "f32" | "f32r" | "bf16"
MM_MODE = os.environ.get("DOCRED_MM_MODE", "f32r")


def _fdt():
    return BF16 if MM_MODE == "bf16" else F32


def _np_fdt():
    import ml_dtypes

    return np.dtype(ml_dtypes.bfloat16) if MM_MODE == "bf16" else np.float32


class _Builder:
    def __init__(self, mm_mode: str):
        self.mm_mode = mm_mode
        self.fdt = {"f32": F32, "f32r": F32R, "bf16": BF16}[mm_mode]
        nc = bacc.Bacc("TRN2", target_bir_lowering=False, debug=False)
        self.nc = nc
        fdt = self.fdt
        # ---- DRAM tensors (per-core inputs) ----
        d = {}
        d["ent"] = nc.dram_tensor("ent", [M, H], F32, kind="ExternalInput")
        d["attn"] = nc.dram_tensor("attn", [M, NH * L], fdt, kind="ExternalInput")
        d["seq"] = nc.dram_tensor("seq", [128, 8 * (L + 1)], fdt, kind="ExternalInput")
        d["ssum"] = nc.dram_tensor("ssum", [M, E], fdt, kind="ExternalInput")
        d["smean"] = nc.dram_tensor("smean", [M, E], fdt, kind="ExternalInput")
        d["eadd"] = nc.dram_tensor("eadd", [E, 1], F32, kind="ExternalInput")
        d["ohx"] = nc.dram_tensor("ohx", [E, RPC], fdt, kind="ExternalInput")
        d["ohy"] = nc.dram_tensor("ohy", [E, RPC], fdt, kind="ExternalInput")
        d["wh"] = nc.dram_tensor("wh", [2 * H, EMB], fdt, kind="ExternalInput")
        d["wt"] = nc.dram_tensor("wt", [2 * H, EMB], fdt, kind="ExternalInput")
        d["bh"] = nc.dram_tensor("bh", [1, EMB], fdt, kind="ExternalInput")
        d["bt"] = nc.dram_tensor("bt", [1, EMB], fdt, kind="ExternalInput")
        d["wb"] = nc.dram_tensor("wb", [128, 384 * NCL], fdt, kind="ExternalInput")
        d["bbc"] = nc.dram_tensor("bbc", [NCL, 1], F32, kind="ExternalInput")
        d["ident"] = nc.dram_tensor("ident", [128, 128], fdt, kind="ExternalInput")
        d["ones1"] = nc.dram_tensor("ones1", [1, 128], fdt, kind="ExternalInput")
        d["lt"] = nc.dram_tensor("lt", [NCL, RPC], F32, kind="ExternalOutput")
        self.d = d
        with tile.TileContext(nc) as tc:
            self.build(tc)
        nc.compile()

    def mm(self, out, lhsT, rhs, **kw):
        return self.nc.tensor.matmul(out, lhsT, rhs, **kw)

    def tp(self, out, in_, ident, **kw):
        return self.nc.tensor.matmul(out, in_, ident, is_transpose=True, **kw)

    def build(self, tc):
        nc = self.nc
        d = self.d
        fdt = self.fdt
        AF = mybir.ActivationFunctionType

        with (
            tc.tile_pool(name="pin", bufs=1) as pin,
            tc.tile_pool(name="mid", bufs=1) as mid,
            tc.tile_pool(name="late", bufs=1) as late,
            tc.tile_pool(name="ahpool", bufs=3) as ahpool,
            tc.tile_pool(name="wstream", bufs=2) as wstream,
            tc.tile_pool(name="wbstream", bufs=2) as wbstream,
            tc.tile_pool(name="blpool", bufs=2) as blpool,
            tc.tile_pool(name="bltpool", bufs=2) as bltpool,
            tc.tile_pool(name="gx", bufs=3) as gxpool,
            tc.tile_pool(name="ps_sm", bufs=2, space="PSUM") as ps_sm,
        ):
            # ---------- load small persistent tensors ----------
            ssum = pin.tile([M, E], fdt)
            smean = pin.tile([M, E], fdt)
            eadd = pin.tile([E, 1], F32)
            ohx = pin.tile([E, RPC], fdt)
            ohy = pin.tile([E, RPC], fdt)
            ident = pin.tile([128, 128], fdt)
            bh = pin.tile([1, EMB], fdt)
            bt = pin.tile([1, EMB], fdt)
            bbc = pin.tile([NCL, 1], F32)
            ones1 = pin.tile([1, 128], fdt)
            for t, key in [
                (ssum, "ssum"), (smean, "smean"), (eadd, "eadd"),
                (ohx, "ohx"), (ohy, "ohy"), (ident, "ident"),
                (bh, "bh"), (bt, "bt"), (bbc, "bbc"), (ones1, "ones1"),
            ]:
                nc.sync.dma_start(t[:], d[key].ap())

            with (
                tc.tile_pool(name="ps_big", bufs=2, space="PSUM") as ps_big,
                tc.tile_pool(name="ps_ss", bufs=2, space="PSUM") as ps_ss,
            ):
                # ---------- P1: exp + segment-sum + log ----------
                ent = mid.tile([M, H], F32)
                nc.sync.dma_start(ent[:], d["ent"].ap())
                if self.mm_mode != "f32":
                    pexp = mid.tile([M, H], fdt, name="pexp")
                else:
                    pexp = ent
                nc.scalar.activation(pexp[:], ent[:], AF.Exp)
                ps_ent = ps_big.tile([E, H], F32, tag="big")
                for nh in range(2):
                    self.mm(ps_ent[:, nh * 512:(nh + 1) * 512], ssum[:],
                            pexp[:, nh * 512:(nh + 1) * 512])
                ent_sb = mid.tile([E, H], fdt)
                nc.scalar.activation(ent_sb[:], ps_ent[:], AF.Ln, bias=eadd[:])

                # ---------- P2: hs/ts gathers (ent_sb^T gathered by pair) ----
                hsT = mid.tile([128, 8, RPC], fdt)
                tsT = mid.tile([128, 8, RPC], fdt)
                for hc in range(8):
                    for dst, oh in ((hsT, ohx), (tsT, ohy)):
                        ps_g = ps_sm.tile([128, RPC], F32, tag="sm")
                        self.mm(ps_g[:], ent_sb[:, hc * 128:(hc + 1) * 128], oh[:])
                        nc.scalar.copy(dst[:, hc, :], ps_g[:])

                # ---------- P3: attn pooling fused with C accumulation ------
                # C^T[l, r] = sum_h Ax_h[l, r] * Ay_h[l, r]
                CT = mid.tile([128, 8, RPC], F32)
                CTmm = (CT if self.mm_mode == "f32"
                        else mid.tile([128, 8, RPC], fdt, name="CTmm"))
                attn_view = d["attn"].ap().rearrange("p (h l) -> p h l", h=NH)
                for hpair in range(NH // 2):
                    at = ahpool.tile([M, 2, L], fdt, tag="attn_in", bufs=2)
                    nc.sync.dma_start(at[:], attn_view[:, 2 * hpair:2 * hpair + 2, :])
                    for hh in range(2):
                        h = 2 * hpair + hh
                        A_h = ahpool.tile([E, L], fdt, tag="ah")
                        for nh in range(2):
                            ps_a = ps_sm.tile([E, 512], F32, tag="sm")
                            self.mm(ps_a[:], smean[:],
                                    at[:, hh, nh * 512:(nh + 1) * 512])
                            nc.scalar.copy(A_h[:, nh * 512:(nh + 1) * 512], ps_a[:])
                        for lc in range(8):
                            ps_x = ps_sm.tile([128, RPC], F32, tag="sm")
                            ps_y = ps_sm.tile([128, RPC], F32, tag="sm")
                            a_sl = A_h[:, lc * 128:(lc + 1) * 128]
                            self.mm(ps_x[:], a_sl, ohx[:])
                            self.mm(ps_y[:], a_sl, ohy[:])
                            gxs = gxpool.tile([128, RPC], fdt, tag="gxs", bufs=2)
                            nc.scalar.copy(gxs[:], ps_x[:])
                            if h == 0:
                                nc.vector.tensor_mul(CT[:, lc, :], gxs[:], ps_y[:])
                            else:
                                tmp = gxpool.tile([128, RPC], F32, tag="ctmp", bufs=2)
                                nc.vector.tensor_mul(tmp[:], gxs[:], ps_y[:])
                                out_t = CT if (h < NH - 1 or CTmm is CT) else CTmm
                                nc.vector.tensor_add(out_t[:, lc, :],
                                                     CT[:, lc, :], tmp[:])

                # ---------- P4: rel = (C @ [seq|1]), normalize, transpose ----
                relT = mid.tile([128, 8, RPC], fdt)
                ps_rel = [ps_big.tile([128, L], F32, tag="big", name=f"ps_rel{i}")
                          for i in range(2)]
                ps_s = [ps_ss.tile([128, 1], F32, tag="ss", name=f"ps_s{i}")
                        for i in range(2)]
                seq_view = d["seq"].ap().rearrange("p (a b) -> p a b", a=8)
                for lc in range(8):
                    sq = ahpool.tile([128, L + 1], fdt, tag="sq")
                    nc.sync.dma_start(sq[:], seq_view[:, lc, :])
                    st, sp = lc == 0, lc == 7
                    for rc in range(2):
                        lhsT = CTmm[:, lc, rc * 128:(rc + 1) * 128]
                        self.mm(ps_rel[rc][:, 0:512], lhsT, sq[:, 0:512],
                                start=st, stop=sp)
                        self.mm(ps_rel[rc][:, 512:1024], lhsT, sq[:, 512:1024],
                                start=st, stop=sp)
                        if self.mm_mode == "f32r":
                            # fp32r ISA rejects free-dim-1 matmuls
                            self.mm(ps_s[rc][:], lhsT.bitcast(F32),
                                    sq[:, 1024:1025].bitcast(F32),
                                    start=st, stop=sp)
                        else:
                            self.mm(ps_s[rc][:], lhsT, sq[:, 1024:1025],
                                    start=st, stop=sp)
                for rc in range(2):
                    tdenom = gxpool.tile([128, 1], F32, tag="tden")
                    nc.scalar.activation(tdenom[:], ps_s[rc][:], AF.Copy,
                                         bias=16e-5, scale=1.0)
                    frec = gxpool.tile([128, 1], F32, tag="frec")
                    nc.vector.reciprocal(frec[:], tdenom[:])
                    rel_sc = mid.tile([128, L], fdt, tag="rel_sc", name="rel_sc")
                    nc.vector.tensor_scalar_mul(rel_sc[:], ps_rel[rc][:], frec[:])
                    for dc in range(8):
                        ps_t = ps_sm.tile([128, 128], fdt, tag="sm")
                        self.tp(ps_t[:], rel_sc[:, dc * 128:(dc + 1) * 128], ident[:])
                        nc.scalar.copy(relT[:, dc, rc * 128:(rc + 1) * 128], ps_t[:])

                # ---------- P5: extractors -> hsE/tsE [n, EMB] ---------------
                hsE = late.tile([128, 2, EMB], fdt)
                tsE = late.tile([128, 2, EMB], fdt)
                for (wkey, bvec, xT, dst) in (
                    ("wh", bh, hsT, hsE), ("wt", bt, tsT, tsE),
                ):
                    ps_e = [ps_big.tile([128, EMB], F32, tag="big",
                                        name=f"ps_e{i}") for i in range(2)]
                    for kc in range(16):
                        w = wstream.tile([128, EMB], fdt, tag="w")
                        nc.sync.dma_start(w[:],
                                          d[wkey].ap()[kc * 128:(kc + 1) * 128, :])
                        src = xT if kc < 8 else relT
                        for rc in range(2):
                            lhsT = src[:, kc % 8, rc * 128:(rc + 1) * 128]
                            for lo, hi in ((0, 512), (512, 768)):
                                self.mm(ps_e[rc][:, lo:hi], lhsT, w[:, lo:hi],
                                        start=(kc == 0), stop=False)
                    for rc in range(2):
                        for lo, hi in ((0, 512), (512, 768)):
                            self.mm(ps_e[rc][:, lo:hi], ones1[:], bvec[:, lo:hi],
                                    start=False, stop=True)
                        nc.scalar.activation(dst[:, rc, :], ps_e[rc][:], AF.Tanh)

            # ---------- P6: block bilinear + classifier ----------
            with tc.tile_pool(name="ps_lt", bufs=1, space="PSUM") as ps_lt:
                pslt = ps_lt.tile([NCL, RPC], F32)
                for k in range(K12):
                    for half in range(2):
                        wb = wbstream.tile([128, 16 * NCL], fdt, tag="wb")
                        nc.sync.dma_start(
                            wb[:], d["wb"].ap()
                            [:, (k * 32 + half * 16) * NCL:
                                (k * 32 + (half + 1) * 16) * NCL])
                        bl = blpool.tile([128, 2, BS // 2, BS], fdt, tag="bl")
                        for rc in range(2):
                            i0 = half * (BS // 2)
                            b1 = hsE[:, rc, k * BS + i0: k * BS + i0 + BS // 2]
                            b2 = tsE[:, rc, k * BS:(k + 1) * BS]
                            b1b = b1.unsqueeze(2).broadcast_to(
                                [128, BS // 2, BS])
                            b2b = b2.unsqueeze(1).broadcast_to(
                                [128, BS // 2, BS])
                            nc.vector.tensor_mul(bl[:, rc], b1b, b2b)
                        blT = bltpool.tile([128, 16, 2, 128], fdt, tag="blT")
                        for c in range(16):
                            for rc in range(2):
                                ps_t = ps_sm.tile([128, 128], fdt, tag="sm")
                                self.tp(ps_t[:],
                                        bl[:, rc].rearrange("p a b -> p (a b)")
                                        [:, c * 128:(c + 1) * 128],
                                        ident[:])
                                nc.scalar.copy(blT[:, c, rc, :], ps_t[:])
                        for c in range(16):
                            cg = k * 32 + half * 16 + c
                            self.mm(pslt[:], wb[:, c * NCL:(c + 1) * NCL],
                                    blT[:, c].rearrange("p a b -> p (a b)"),
                                    start=(cg == 0), stop=(cg == 383))

                out_sb = late.tile([NCL, RPC], F32)
                nc.scalar.activation(out_sb[:], pslt[:], AF.Identity, bias=bbc[:])
                nc.sync.dma_start(d["lt"].ap(), out_sb[:])


_PROGRAM_CACHE = {}


def _get_program(mm_mode: str):
    if mm_mode not in _PROGRAM_CACHE:
        _PROGRAM_CACHE[mm_mode] = _Builder(mm_mode)
    return _PROGRAM_CACHE[mm_mode]


def _host_inputs(seq_lhs, ent_lhs, ent_to_seq_attn, entity_id_labels, hts,
                 Wh, bh, Wt, bt, Wb, bb):
    """Build the 8 per-core input maps (all host-side numpy)."""
    fdt = _np_fdt()
    seq_lhs = np.asarray(seq_lhs, np.float32)
    ent_lhs = np.asarray(ent_lhs, np.float32)
    ent_to_seq_attn = np.asarray(ent_to_seq_attn, np.float32)
    entity_id_labels = np.asarray(entity_id_labels)
    hts = np.asarray(hts)
    Wh = np.asarray(Wh, np.float32)
    Wt = np.asarray(Wt, np.float32)
    Wb = np.asarray(Wb, np.float32)
    bh = np.asarray(bh, np.float32)
    bt = np.asarray(bt, np.float32)
    bb = np.asarray(bb, np.float32)

    wb_r = np.ascontiguousarray(
        Wb.reshape(384, 128, NCL).transpose(1, 0, 2).reshape(128, 384 * NCL)
    ).astype(fdt)
    wh_c = Wh.astype(fdt)
    wt_c = Wt.astype(fdt)
    bh_c = bh.reshape(1, EMB).astype(fdt)
    bt_c = bt.reshape(1, EMB).astype(fdt)
    bb_c = np.ascontiguousarray(bb.reshape(NCL, 1))
    ident = np.eye(128, dtype=np.float32).astype(fdt)

    in_maps = []
    for c in range(NCORES):
        doc, half = divmod(c, 2)
        sl = slice(half * RPC, (half + 1) * RPC)
        labels = entity_id_labels[doc].astype(np.int64)
        cnt = np.bincount(labels, minlength=E).astype(np.float32)
        S = np.zeros((M, E), np.float32)
        S[np.arange(M), labels] = 1.0
        smean = S / np.maximum(cnt, 1.0)[None, :]
        eadd = (cnt == 0).astype(np.float32).reshape(E, 1)
        hi = hts[doc, sl, 0].astype(np.int64)
        ti = hts[doc, sl, 1].astype(np.int64)
        ohx = np.zeros((E, RPC), np.float32)
        ohx[hi, np.arange(RPC)] = 1.0
        ohy = np.zeros((E, RPC), np.float32)
        ohy[ti, np.arange(RPC)] = 1.0
        attn = np.ascontiguousarray(
            ent_to_seq_attn[doc].transpose(1, 0, 2).reshape(M, NH * L)
        ).astype(fdt)
        seq_r = seq_lhs[doc].reshape(8, 128, L).transpose(1, 0, 2)
        seq_aug = np.concatenate(
            [seq_r, np.ones((128, 8, 1), np.float32)], axis=2
        )
        in_maps.append({
            "ent": np.ascontiguousarray(ent_lhs[doc]),
            "attn": attn,
            "seq": np.ascontiguousarray(seq_aug.reshape(128, 8 * (L + 1))).astype(fdt),
            "ssum": S.astype(fdt),
            "smean": smean.astype(fdt),
            "eadd": eadd,
            "ohx": ohx.astype(fdt),
            "ohy": ohy.astype(fdt),
            "wh": wh_c, "wt": wt_c, "bh": bh_c, "bt": bt_c,
            "wb": wb_r, "bbc": bb_c, "ident": ident,
            "ones1": np.ones((1, 128), np.float32).astype(fdt),
        })
    return in_maps


_LAST_RESULTS = {}


def kernel(**inputs) -> np.ndarray:
    prog = _get_program(MM_MODE)
    in_maps = _host_inputs(**inputs)
    trace = os.environ.get("DOCRED_TRACE", "0") == "1"
    res = run_bass_kernel_spmd(
        prog.nc, in_maps, core_ids=list(range(NCORES)), trace=trace,
    )
    _LAST_RESULTS["res"] = res
    out = np.empty((B * R, NCL), np.float32)
    for c in range(NCORES):
        doc, half = divmod(c, 2)
        lt = res.results[c]["lt"]  # [NCL, RPC]
        out[doc * R + half * RPC: doc * R + (half + 1) * RPC, :] = lt.T
    return out


# revision 21
# speedup vs baseline: 1.8235x; 1.8235x over previous
"""Trainium2 Bass kernel for the DocRED-style segment_reduce model.

Sharding: 8 cores, data-parallel: core c -> (doc = c//2, pair-half = c%2).
Each core independently computes logits for its 256 pairs. No collectives.
All segment reductions / gathers are lowered to one-hot matmuls whose
one-hot matrices are built on the host from the integer inputs and passed
as per-core input tensors (the SPMD program itself is index-agnostic).
"""

import os

import numpy as np

import concourse.bacc as bacc
import concourse.bass as bass
import concourse.mybir as mybir
import concourse.tile as tile
from concourse.bass_utils import run_bass_kernel_spmd

B, M, H = 4, 128, 1024
NH, L = 16, 1024
E, R = 64, 512
EMB, BS, NCL = 768, 64, 97
K12 = EMB // BS  # 12 blocks
NCORES = 8
RPC = R // 2  # pairs per core

F32 = mybir.dt.float32
F32R = mybir.dt.float32r
BF16 = mybir.dt.bfloat16

# matmul/compute dtype mode: "f32" | "f32r" | "bf16"
MM_MODE = os.environ.get("DOCRED_MM_MODE", "bf16")


def _fdt():
    return BF16 if MM_MODE == "bf16" else F32


def _np_fdt():
    import ml_dtypes

    return np.dtype(ml_dtypes.bfloat16) if MM_MODE == "bf16" else np.float32


class _Builder:
    def __init__(self, mm_mode: str):
        self.mm_mode = mm_mode
        self.fdt = {"f32": F32, "f32r": F32R, "bf16": BF16}[mm_mode]
        nc = bacc.Bacc("TRN2", target_bir_lowering=False, debug=False)
        self.nc = nc
        fdt = self.fdt
        # ---- DRAM tensors (per-core inputs) ----
        d = {}
        d["ent"] = nc.dram_tensor("ent", [M, H], F32, kind="ExternalInput")
        d["attn"] = nc.dram_tensor("attn", [M, NH * L], fdt, kind="ExternalInput")
        d["seq"] = nc.dram_tensor("seq", [128, 8 * (L + 1)], fdt, kind="ExternalInput")
        d["ssum"] = nc.dram_tensor("ssum", [M, E], fdt, kind="ExternalInput")
        d["smean"] = nc.dram_tensor("smean", [M, E], fdt, kind="ExternalInput")
        d["eadd"] = nc.dram_tensor("eadd", [E, 1], F32, kind="ExternalInput")
        d["ohx"] = nc.dram_tensor("ohx", [E, RPC], fdt, kind="ExternalInput")
        d["ohy"] = nc.dram_tensor("ohy", [E, RPC], fdt, kind="ExternalInput")
        d["wh"] = nc.dram_tensor("wh", [2 * H, EMB], fdt, kind="ExternalInput")
        d["wt"] = nc.dram_tensor("wt", [2 * H, EMB], fdt, kind="ExternalInput")
        d["bh"] = nc.dram_tensor("bh", [1, EMB], fdt, kind="ExternalInput")
        d["bt"] = nc.dram_tensor("bt", [1, EMB], fdt, kind="ExternalInput")
        d["wb"] = nc.dram_tensor("wb", [128, 384 * NCL], fdt, kind="ExternalInput")
        d["bbc"] = nc.dram_tensor("bbc", [NCL, 1], F32, kind="ExternalInput")
        d["ident"] = nc.dram_tensor("ident", [128, 128], fdt, kind="ExternalInput")
        d["ones1"] = nc.dram_tensor("ones1", [1, 128], fdt, kind="ExternalInput")
        d["lt"] = nc.dram_tensor("lt", [NCL, RPC], F32, kind="ExternalOutput")
        self.d = d
        with tile.TileContext(nc) as tc:
            self.build(tc)
        nc.compile()

    def mm(self, out, lhsT, rhs, **kw):
        return self.nc.tensor.matmul(out, lhsT, rhs, **kw)

    def tp(self, out, in_, ident, **kw):
        return self.nc.tensor.matmul(out, in_, ident, is_transpose=True, **kw)

    def build(self, tc):
        nc = self.nc
        d = self.d
        fdt = self.fdt
        AF = mybir.ActivationFunctionType

        with (
            tc.tile_pool(name="pin", bufs=1) as pin,
            tc.tile_pool(name="mid", bufs=1) as mid,
            tc.tile_pool(name="late", bufs=1) as late,
            tc.tile_pool(name="ahpool", bufs=3) as ahpool,
            tc.tile_pool(name="wstream", bufs=2) as wstream,
            tc.tile_pool(name="wbstream", bufs=2) as wbstream,
            tc.tile_pool(name="blpool", bufs=2) as blpool,
            tc.tile_pool(name="bltpool", bufs=2) as bltpool,
            tc.tile_pool(name="gx", bufs=3) as gxpool,
            tc.tile_pool(name="ps_sm", bufs=2, space="PSUM") as ps_sm,
        ):
            # ---------- load small persistent tensors ----------
            ssum = pin.tile([M, E], fdt)
            smean = pin.tile([M, E], fdt)
            eadd = pin.tile([E, 1], F32)
            ohx = pin.tile([E, RPC], fdt)
            ohy = pin.tile([E, RPC], fdt)
            ident = pin.tile([128, 128], fdt)
            bh = pin.tile([1, EMB], fdt)
            bt = pin.tile([1, EMB], fdt)
            bbc = pin.tile([NCL, 1], F32)
            ones1 = pin.tile([1, 128], fdt)
            for t, key in [
                (ssum, "ssum"), (smean, "smean"), (eadd, "eadd"),
                (ohx, "ohx"), (ohy, "ohy"), (ident, "ident"),
                (bh, "bh"), (bt, "bt"), (bbc, "bbc"), (ones1, "ones1"),
            ]:
                nc.sync.dma_start(t[:], d[key].ap())

            with tc.tile_pool(name="ps_big", bufs=2, space="PSUM") as ps_big:
                # ---------- P1: exp + segment-sum + log ----------
                ent = mid.tile([M, H], F32)
                nc.sync.dma_start(ent[:], d["ent"].ap())
                if self.mm_mode != "f32":
                    pexp = mid.tile([M, H], fdt, name="pexp")
                else:
                    pexp = ent
                nc.scalar.activation(pexp[:], ent[:], AF.Exp)
                ps_ent = ps_big.tile([E, H], F32, tag="big")
                for nh in range(2):
                    self.mm(ps_ent[:, nh * 512:(nh + 1) * 512], ssum[:],
                            pexp[:, nh * 512:(nh + 1) * 512])
                ent_sb = mid.tile([E, H], fdt)
                nc.scalar.activation(ent_sb[:], ps_ent[:], AF.Ln, bias=eadd[:])

                # ---------- P2: hs/ts gathers (ent_sb^T gathered by pair) ----
                hsT = mid.tile([128, 8, RPC], fdt)
                tsT = mid.tile([128, 8, RPC], fdt)
                for hc in range(8):
                    for dst, oh in ((hsT, ohx), (tsT, ohy)):
                        ps_g = ps_sm.tile([128, RPC], F32, tag="sm")
                        self.mm(ps_g[:], ent_sb[:, hc * 128:(hc + 1) * 128], oh[:])
                        nc.scalar.copy(dst[:, hc, :], ps_g[:])

                # ---------- P3: attn pooling fused with C accumulation ------
                # C^T[l, r] = sum_h Ax_h[l, r] * Ay_h[l, r], 2 heads per pass
                CT = mid.tile([128, 8, RPC], F32)
                CTmm = (CT if self.mm_mode == "f32"
                        else mid.tile([128, 8, RPC], fdt, name="CTmm"))
                attn_view = d["attn"].ap().rearrange("p (h l) -> p h l", h=NH)
                with tc.tile_pool(name="ps_pl", bufs=2, space="PSUM") as ps_pl:
                    for g in range(NH // 2):
                        at = ahpool.tile([M, 2, L], fdt, tag="attn_in", bufs=2)
                        nc.sync.dma_start(at[:],
                                          attn_view[:, 2 * g:2 * g + 2, :])
                        A_h2 = ahpool.tile([E, 2, L], fdt, tag="ah")
                        for hh in range(2):
                            for nh in range(2):
                                ps_a = ps_pl.tile([E, 512], F32, tag="pl")
                                self.mm(ps_a[:], smean[:],
                                        at[:, hh, nh * 512:(nh + 1) * 512])
                                nc.scalar.copy(
                                    A_h2[:, hh, nh * 512:(nh + 1) * 512], ps_a[:])
                        for lc in range(8):
                            ps_x2 = ps_sm.tile([128, 2, RPC], F32, tag="sm")
                            ps_y2 = ps_sm.tile([128, 2, RPC], F32, tag="sm")
                            for hh in range(2):
                                a_sl = A_h2[:, hh, lc * 128:(lc + 1) * 128]
                                self.mm(ps_x2[:, hh, :], a_sl, ohx[:])
                                self.mm(ps_y2[:, hh, :], a_sl, ohy[:])
                            gxs2 = gxpool.tile([128, 2, RPC], fdt, tag="gxs",
                                               bufs=2)
                            nc.scalar.copy(gxs2[:], ps_x2[:])
                            tmp2 = gxpool.tile([128, 2, RPC], fdt, tag="ctmp",
                                               bufs=2)
                            nc.vector.tensor_mul(tmp2[:], gxs2[:], ps_y2[:])
                            if g == 0:
                                nc.vector.tensor_add(CT[:, lc, :], tmp2[:, 0, :],
                                                     tmp2[:, 1, :])
                            else:
                                nc.vector.tensor_add(CT[:, lc, :], CT[:, lc, :],
                                                     tmp2[:, 0, :])
                                out_t = CT if (g < NH // 2 - 1 or CTmm is CT) \
                                    else CTmm
                                nc.vector.tensor_add(out_t[:, lc, :], CT[:, lc, :],
                                                     tmp2[:, 1, :])

                # ---------- P4: rel = (C @ [seq|1]), normalize, transpose ----
                ps_ss_cm = tc.tile_pool(name="ps_ss", bufs=2, space="PSUM")
                ps_ss = ps_ss_cm.__enter__()
                relT = mid.tile([128, 8, RPC], fdt)
                ps_rel = [ps_big.tile([128, L], F32, tag="big", name=f"ps_rel{i}")
                          for i in range(2)]
                ps_s = [ps_ss.tile([128, 1], F32, tag="ss", name=f"ps_s{i}")
                        for i in range(2)]
                seq_view = d["seq"].ap().rearrange("p (a b) -> p a b", a=8)
                for lc in range(8):
                    sq = ahpool.tile([128, L + 1], fdt, tag="sq")
                    nc.sync.dma_start(sq[:], seq_view[:, lc, :])
                    st, sp = lc == 0, lc == 7
                    for rc in range(2):
                        lhsT = CTmm[:, lc, rc * 128:(rc + 1) * 128]
                        self.mm(ps_rel[rc][:, 0:512], lhsT, sq[:, 0:512],
                                start=st, stop=sp)
                        self.mm(ps_rel[rc][:, 512:1024], lhsT, sq[:, 512:1024],
                                start=st, stop=sp)
                        if self.mm_mode == "f32r":
                            # fp32r ISA rejects free-dim-1 matmuls
                            self.mm(ps_s[rc][:], lhsT.bitcast(F32),
                                    sq[:, 1024:1025].bitcast(F32),
                                    start=st, stop=sp)
                        else:
                            self.mm(ps_s[rc][:], lhsT, sq[:, 1024:1025],
                                    start=st, stop=sp)
                for rc in range(2):
                    tdenom = gxpool.tile([128, 1], F32, tag="tden")
                    nc.scalar.activation(tdenom[:], ps_s[rc][:], AF.Copy,
                                         bias=16e-5, scale=1.0)
                    frec = gxpool.tile([128, 1], F32, tag="frec")
                    nc.vector.reciprocal(frec[:], tdenom[:])
                    rel_sc = mid.tile([128, L], fdt, tag="rel_sc", name="rel_sc")
                    nc.vector.tensor_scalar_mul(rel_sc[:], ps_rel[rc][:], frec[:])
                    for dc in range(8):
                        ps_t = ps_sm.tile([128, 128], fdt, tag="sm")
                        self.tp(ps_t[:], rel_sc[:, dc * 128:(dc + 1) * 128], ident[:])
                        nc.scalar.copy(relT[:, dc, rc * 128:(rc + 1) * 128], ps_t[:])

                # ---------- P5: extractors -> hsE/tsE [n, EMB] ---------------
                hsE = late.tile([128, 2, EMB], fdt)
                tsE = late.tile([128, 2, EMB], fdt)
                for (wkey, bvec, xT, dst) in (
                    ("wh", bh, hsT, hsE), ("wt", bt, tsT, tsE),
                ):
                    ps_e = [ps_big.tile([128, EMB], F32, tag="big",
                                        name=f"ps_e{i}") for i in range(2)]
                    for kc in range(16):
                        w = wstream.tile([128, EMB], fdt, tag="w")
                        nc.sync.dma_start(w[:],
                                          d[wkey].ap()[kc * 128:(kc + 1) * 128, :])
                        src = xT if kc < 8 else relT
                        for rc in range(2):
                            lhsT = src[:, kc % 8, rc * 128:(rc + 1) * 128]
                            for lo, hi in ((0, 512), (512, 768)):
                                self.mm(ps_e[rc][:, lo:hi], lhsT, w[:, lo:hi],
                                        start=(kc == 0), stop=False)
                    for rc in range(2):
                        for lo, hi in ((0, 512), (512, 768)):
                            self.mm(ps_e[rc][:, lo:hi], ones1[:], bvec[:, lo:hi],
                                    start=False, stop=True)
                        nc.scalar.activation(dst[:, rc, :], ps_e[rc][:], AF.Tanh)
                ps_ss_cm.__exit__(None, None, None)

            # ---------- P6: block bilinear + classifier ----------
            with tc.tile_pool(name="ps_lt", bufs=1, space="PSUM") as ps_lt:
                pslt = ps_lt.tile([NCL, RPC], F32)
                for k in range(K12):
                    for half in range(2):
                        wb = wbstream.tile([128, 16 * NCL], fdt, tag="wb")
                        nc.sync.dma_start(
                            wb[:], d["wb"].ap()
                            [:, (k * 32 + half * 16) * NCL:
                                (k * 32 + (half + 1) * 16) * NCL])
                        bl = blpool.tile([128, 2, BS // 2, BS], fdt, tag="bl")
                        for rc in range(2):
                            i0 = half * (BS // 2)
                            b1 = hsE[:, rc, k * BS + i0: k * BS + i0 + BS // 2]
                            b2 = tsE[:, rc, k * BS:(k + 1) * BS]
                            b1b = b1.unsqueeze(2).broadcast_to(
                                [128, BS // 2, BS])
                            b2b = b2.unsqueeze(1).broadcast_to(
                                [128, BS // 2, BS])
                            eng = (nc.gpsimd if (2 * k + half) % 3 == 2
                                   else nc.vector)
                            eng.tensor_mul(bl[:, rc], b1b, b2b)
                        blT = bltpool.tile([128, 16, 2, 128], fdt, tag="blT")
                        for rc in range(2):
                            for cq in range(4):
                                ps4 = ps_sm.tile([128, 4, 128], fdt, tag="sm")
                                for i in range(4):
                                    c = cq * 4 + i
                                    self.tp(ps4[:, i, :],
                                            bl[:, rc].rearrange("p a b -> p (a b)")
                                            [:, c * 128:(c + 1) * 128],
                                            ident[:])
                                nc.scalar.copy(blT[:, cq * 4:(cq + 1) * 4, rc, :],
                                               ps4[:])
                        for c in range(16):
                            cg = k * 32 + half * 16 + c
                            self.mm(pslt[:], wb[:, c * NCL:(c + 1) * NCL],
                                    blT[:, c].rearrange("p a b -> p (a b)"),
                                    start=(cg == 0), stop=(cg == 383))

                out_sb = late.tile([NCL, RPC], F32)
                nc.scalar.activation(out_sb[:], pslt[:], AF.Identity, bias=bbc[:])
                nc.sync.dma_start(d["lt"].ap(), out_sb[:])


_PROGRAM_CACHE = {}


def _get_program(mm_mode: str):
    if mm_mode not in _PROGRAM_CACHE:
        _PROGRAM_CACHE[mm_mode] = _Builder(mm_mode)
    return _PROGRAM_CACHE[mm_mode]


def _host_inputs(seq_lhs, ent_lhs, ent_to_seq_attn, entity_id_labels, hts,
                 Wh, bh, Wt, bt, Wb, bb):
    """Build the 8 per-core input maps (all host-side numpy)."""
    fdt = _np_fdt()
    seq_lhs = np.asarray(seq_lhs, np.float32)
    ent_lhs = np.asarray(ent_lhs, np.float32)
    ent_to_seq_attn = np.asarray(ent_to_seq_attn, np.float32)
    entity_id_labels = np.asarray(entity_id_labels)
    hts = np.asarray(hts)
    Wh = np.asarray(Wh, np.float32)
    Wt = np.asarray(Wt, np.float32)
    Wb = np.asarray(Wb, np.float32)
    bh = np.asarray(bh, np.float32)
    bt = np.asarray(bt, np.float32)
    bb = np.asarray(bb, np.float32)

    wb_r = np.ascontiguousarray(
        Wb.reshape(384, 128, NCL).transpose(1, 0, 2).reshape(128, 384 * NCL)
    ).astype(fdt)
    wh_c = Wh.astype(fdt)
    wt_c = Wt.astype(fdt)
    bh_c = bh.reshape(1, EMB).astype(fdt)
    bt_c = bt.reshape(1, EMB).astype(fdt)
    bb_c = np.ascontiguousarray(bb.reshape(NCL, 1))
    ident = np.eye(128, dtype=np.float32).astype(fdt)

    in_maps = []
    for c in range(NCORES):
        doc, half = divmod(c, 2)
        sl = slice(half * RPC, (half + 1) * RPC)
        labels = entity_id_labels[doc].astype(np.int64)
        cnt = np.bincount(labels, minlength=E).astype(np.float32)
        S = np.zeros((M, E), np.float32)
        S[np.arange(M), labels] = 1.0
        smean = S / np.maximum(cnt, 1.0)[None, :]
        eadd = (cnt == 0).astype(np.float32).reshape(E, 1)
        hi = hts[doc, sl, 0].astype(np.int64)
        ti = hts[doc, sl, 1].astype(np.int64)
        ohx = np.zeros((E, RPC), np.float32)
        ohx[hi, np.arange(RPC)] = 1.0
        ohy = np.zeros((E, RPC), np.float32)
        ohy[ti, np.arange(RPC)] = 1.0
        attn = np.ascontiguousarray(
            ent_to_seq_attn[doc].transpose(1, 0, 2).reshape(M, NH * L)
        ).astype(fdt)
        seq_r = seq_lhs[doc].reshape(8, 128, L).transpose(1, 0, 2)
        seq_aug = np.concatenate(
            [seq_r, np.ones((128, 8, 1), np.float32)], axis=2
        )
        in_maps.append({
            "ent": np.ascontiguousarray(ent_lhs[doc]),
            "attn": attn,
            "seq": np.ascontiguousarray(seq_aug.reshape(128, 8 * (L + 1))).astype(fdt),
            "ssum": S.astype(fdt),
            "smean": smean.astype(fdt),
            "eadd": eadd,
            "ohx": ohx.astype(fdt),
            "ohy": ohy.astype(fdt),
            "wh": wh_c, "wt": wt_c, "bh": bh_c, "bt": bt_c,
            "wb": wb_r, "bbc": bb_c, "ident": ident,
            "ones1": np.ones((1, 128), np.float32).astype(fdt),
        })
    return in_maps


_LAST_RESULTS = {}


def kernel(**inputs) -> np.ndarray:
    prog = _get_program(MM_MODE)
    in_maps = _host_inputs(**inputs)
    trace = os.environ.get("DOCRED_TRACE", "0") == "1"
    res = run_bass_kernel_spmd(
        prog.nc, in_maps, core_ids=list(range(NCORES)), trace=trace,
    )
    _LAST_RESULTS["res"] = res
    out = np.empty((B * R, NCL), np.float32)
    for c in range(NCORES):
        doc, half = divmod(c, 2)
        lt = res.results[c]["lt"]  # [NCL, RPC]
        out[doc * R + half * RPC: doc * R + (half + 1) * RPC, :] = lt.T
    return out
